# revision 17
# baseline (speedup 1.0000x reference)
"""Trainium2 Bass kernel for causal multi-head attention with interleaved RoPE.

Problem: B=2, S=2048, E=2048, H=16, DK=128, fp32, causal, RoPE (interleaved).

Sharding (8 cores): data-parallel over batch (2) x tensor-parallel over head
groups (4 groups of 4 heads). Each core computes, for its (batch b, group g):
    partial_y[S, E] = attn_out_g @ wo[:, g_cols].T
and the host sums the 4 group partials per batch.

Per-core dataflow (all matmuls float32r = full-speed fp32-storage mode):
  - projections in 3 passes (Q, K, V), each pass sb-major with the FULL
    E-contraction accumulated in one PSUM chain (32 matmuls per [128,1024]
    tile) -> a single ACT copy evicts each tile; no DVE eviction adds at
    all.  x is re-DMAed per pass, alternating the scalar/gpsimd queues;
    weights stream on the sync queue.  RoPE (DVE + SBUF-SBUF half-swap
    DMA) runs per s-block right after its eviction.
  - attention per (head, 512-wide q-block), software-pipelined two k-tiles
    deep: scores^T [k,q] on PE into single-bank [128,512] PSUM tiles; causal
    masking via a second accumulating matmul (identity x tri-tile of -1e9)
    so exp(ACT) output needs no post-mask; AV matmuls accumulate on PE while
    the softmax denominator is accumulated OFF the PE by DVE (2/3 of
    k-tiles) and Pool (1/3) elementwise adds, reduced at head end by two
    tiny ones-matmuls; normalize with single-op approx reciprocal + mul.
  - output projection interleaved into the next q-block's score stream via a
    deferred-work queue; wo resident in SBUF (loaded once); PSUM->SBUF
    evictions alternate ACT copy / DVE tensor_scalar_add.
"""
import sys

sys.path.insert(0, "/opt/trn_rl_repo")

import numpy as np

from concourse import bacc, mybir, tile
from concourse import tile_utils

dt = mybir.dt
F32R = dt.float32r
F32 = dt.float32

B, S, E = 2, 2048, 2048
H, DK = 16, 128
HPG = 4            # heads per group
HD = HPG * DK      # 512
P = 128
NE = E // P        # 16 e-tiles
NSB = S // 512     # 4 s-blocks
NQB = 4            # q-blocks
SCALE = 1.0 / float(np.sqrt(DK))

_nc_cache = [None]


def _build():
    # the stock 192KB/partition cap is stale; cayman has 208KB usable
    tile_utils.max_sbuf_usage = 207 * 1024

    nc = bacc.Bacc(None, target_bir_lowering=False)

    xQ = nc.dram_tensor("xQ", [P, NE, S], F32R, kind="ExternalInput")
    wqT = nc.dram_tensor("wqT", [E, HD], F32R, kind="ExternalInput")
    wkT = nc.dram_tensor("wkT", [E, HD], F32R, kind="ExternalInput")
    wvT = nc.dram_tensor("wvT", [E, HD], F32R, kind="ExternalInput")
    woT = nc.dram_tensor("woT", [HD, E], F32R, kind="ExternalInput")
    cosT = nc.dram_tensor("cosT", [P, S], F32, kind="ExternalInput")
    sinT = nc.dram_tensor("sinT", [P, S], F32, kind="ExternalInput")
    triT = nc.dram_tensor("triT", [P, 4 * 512], F32R, kind="ExternalInput")
    identT = nc.dram_tensor("identT", [P, P], F32R, kind="ExternalInput")
    onesT = nc.dram_tensor("onesT", [P, P], F32R, kind="ExternalInput")
    y = nc.dram_tensor("y", [S, E], F32, kind="ExternalOutput")

    EXP = mybir.ActivationFunctionType.Exp
    LN = mybir.ActivationFunctionType.Ln

    with tile.TileContext(nc) as tc:
        with tc.tile_pool(name="res", bufs=1) as res:
            # qt/kt layout: [dk, sb*2048 + h*512 + (s % 512)]
            qt = res.tile([P, HPG * S], F32R, tag="qt")
            kt = res.tile([P, HPG * S], F32R, tag="kt")
            # v layout: [s % 128, (s//128)*512 + h*128 + dv]
            vv = res.tile([P, NE * 512], F32R, tag="vv")
            ones = res.tile([P, P], F32R, tag="ones")
            ident = res.tile([P, P], F32R, tag="ident")
            tri = res.tile([P, 4 * 512], F32R, tag="tri")

            wz = res.tile([P, P], F32, tag="wz")
            nc.vector.memset(wz[:], 0.0)
            nc.scalar.dma_start(out=ones[:], in_=onesT[:, :])

            # ------------- projection phase: 3 passes ---------------------
            with tc.tile_pool(name="csp", bufs=1) as csp, \
                 tc.tile_pool(name="wp", bufs=17) as wp, \
                 tc.tile_pool(name="xsp", bufs=5) as xsp, \
                 tc.tile_pool(name="ropep", bufs=3) as ropep, \
                 tc.tile_pool(name="pps", bufs=4, space="PSUM") as pps:

                # prefetch sb0 x-quads for pass-Q before any bulk constants
                xq0 = {}
                for j in range(4):
                    xq0[j] = xsp.tile([P, 4, 512], F32R, tag="xs", name="xq")
                    eng = nc.scalar if (j & 1) == 0 else nc.gpsimd
                    eng.dma_start(out=xq0[j][:], in_=xQ[:, 4 * j:4 * j + 4, 0:512])

                cos_t = csp.tile([P, S], F32, tag="cos")
                sin_t = csp.tile([P, S], F32, tag="sin")
                nc.gpsimd.dma_start(out=cos_t[:], in_=cosT[:, :])
                nc.gpsimd.dma_start(out=sin_t[:], in_=sinT[:, :])
                nc.gpsimd.dma_start(out=ident[:], in_=identT[:, :])
                nc.gpsimd.dma_start(out=tri[:], in_=triT[:, :])

                def rope(sb, tens, h):
                    # tens slice for (sb, h): u <- u*cos + halfswap(u)*sin_signed
                    base = sb * 2048 + h * 512
                    u = tens[:, base:base + 512]
                    csl = slice(sb * 512, (sb + 1) * 512)
                    eng = nc.vector if h % 2 == 0 else nc.gpsimd
                    sw = ropep.tile([P, 512], F32R, tag="rp", name="sw")
                    nc.sync.dma_start(out=sw[0:64, :], in_=u[64:128, :])
                    nc.sync.dma_start(out=sw[64:128, :], in_=u[0:64, :])
                    eng.tensor_mul(out=sw[:], in0=sw[:], in1=sin_t[:, csl])
                    eng.tensor_mul(out=u, in0=u, in1=cos_t[:, csl])
                    eng.tensor_add(out=u, in0=u, in1=sw[:])

                first = [True]

                def qk_pass(wsrc, dest, pre=None):
                    w_t = {}
                    for e in range(NE):
                        w_t[e] = wp.tile([P, HD], F32R, tag="w", name="w_t")
                        nc.sync.dma_start(
                            out=w_t[e][:], in_=wsrc[e * P:(e + 1) * P, :])
                    for sb in range(NSB):
                        if sb == 0 and pre is not None:
                            xq = pre
                        else:
                            xq = {}
                            for j in range(4):
                                xq[j] = xsp.tile([P, 4, 512], F32R, tag="xs",
                                                 name="xq")
                                eng = nc.scalar if (j & 1) == 0 else nc.gpsimd
                                eng.dma_start(
                                    out=xq[j][:],
                                    in_=xQ[:, 4 * j:4 * j + 4,
                                           sb * 512:(sb + 1) * 512])
                        xs_t = {e: xq[e // 4][:, e % 4, :] for e in range(NE)}
                        ps = [pps.tile([P, 1024], F32, tag="ps", name="psqk")
                              for _ in range(2)]
                        if first[0]:
                            first[0] = False
                            for _ in range(24):
                                nc.tensor.matmul(ps[0][0:32, 0:128],
                                                 wz[:, 0:32], wz[:],
                                                 start=True, stop=True)
                        for e in range(NE):
                            st_, sp_ = e == 0, e == NE - 1
                            for h in range(HPG):
                                nc.tensor.matmul(
                                    ps[h // 2][:, (h % 2) * 512:(h % 2) * 512 + 512],
                                    w_t[e][:, h * P:(h + 1) * P],
                                    xs_t[e][:], start=st_, stop=sp_)
                        for hp in range(2):
                            nc.scalar.copy(
                                out=dest[:, sb * 2048 + hp * 1024:
                                         sb * 2048 + hp * 1024 + 1024],
                                in_=ps[hp][:])
                        for h in range(HPG):
                            rope(sb, dest, h)

                qk_pass(wqT, qt, pre=xq0)
                qk_pass(wkT, kt)

                # V pass
                wv_t = {}
                for e in range(NE):
                    wv_t[e] = wp.tile([P, HD], F32R, tag="w", name="wv_t")
                    nc.sync.dma_start(
                        out=wv_t[e][:], in_=wvT[e * P:(e + 1) * P, :])
                for sb in range(NSB):
                    xq = {}
                    for j in range(4):
                        xq[j] = xsp.tile([P, 4, 512], F32R, tag="xs", name="xq")
                        eng = nc.scalar if (j & 1) == 0 else nc.gpsimd
                        eng.dma_start(
                            out=xq[j][:],
                            in_=xQ[:, 4 * j:4 * j + 4,
                                   sb * 512:(sb + 1) * 512])
                    xs_t = {e: xq[e // 4][:, e % 4, :] for e in range(NE)}
                    psv = [pps.tile([P, 1024], F32, tag="ps", name="psv")
                           for _ in range(2)]
                    for e in range(NE):
                        st_, sp_ = e == 0, e == NE - 1
                        for st in range(4):
                            nc.tensor.matmul(
                                psv[st // 2][:, (st % 2) * 512:(st % 2) * 512 + 512],
                                xs_t[e][:, st * P:(st + 1) * P],
                                wv_t[e][:], start=st_, stop=sp_)
                    for sp2 in range(2):
                        gst = sb * 4 + sp2 * 2
                        nc.scalar.copy(out=vv[:, gst * 512:(gst + 2) * 512],
                                       in_=psv[sp2][:])

            # ------------- attention + out-proj phase --------------------
            with tc.tile_pool(name="worp", bufs=1) as worp, \
                 tc.tile_pool(name="atp", bufs=2) as atp, \
                 tc.tile_pool(name="ztp", bufs=4) as ztp, \
                 tc.tile_pool(name="recp", bufs=2) as recp, \
                 tc.tile_pool(name="obp", bufs=4) as obp, \
                 tc.tile_pool(name="aps", bufs=2, space="PSUM") as aps:

                wo_r = []
                for hh in range(HPG):
                    wt = worp.tile([P, E], F32R, tag=f"wo{hh}", name="wt")
                    nc.sync.dma_start(out=wt[:], in_=woT[hh * P:(hh + 1) * P, :])
                    wo_r.append(wt)

                # deferred out-proj tiles, popped between attention items
                deferred = []

                def pop_deferred(n=1):
                    for _ in range(n):
                        if deferred:
                            deferred.pop(0)()

                def emit_outproj(qb, at_t):
                    # 16 psum tiles [128 q, 512 e], each = 4 accumulating MMs
                    def mk(st, e5):
                        def go():
                            ps_o = aps.tile([P, 512], F32, tag="po", name="ps_o")
                            for h in range(HPG):
                                nc.tensor.matmul(
                                    ps_o[:],
                                    at_t[:, h * 512 + st * P: h * 512 + (st + 1) * P],
                                    wo_r[h][:, e5 * 512:(e5 + 1) * 512],
                                    start=(h == 0), stop=(h == HPG - 1))
                            ob = obp.tile([P, 512], F32, tag="ob", name="ob")
                            nc.vector.tensor_scalar_add(ob[:], ps_o[:], 0.0)
                            srow = qb * 512 + st * P
                            nc.sync.dma_start(
                                out=y[srow:srow + P, e5 * 512:(e5 + 1) * 512],
                                in_=ob[:])
                        return go
                    for st in range(4):
                        for e5 in range(4):
                            deferred.append(mk(st, e5))

                def sc_mm(qb, h, kt_i, ps_s):
                    # scores^T for one 128-wide k-tile, plus causal tri-mask
                    sbk, r = divmod(kt_i, 4)
                    qsl = qt[:, qb * 2048 + h * 512: qb * 2048 + (h + 1) * 512]
                    diag = kt_i >= qb * 4
                    nc.tensor.matmul(
                        ps_s[:],
                        kt[:, sbk * 2048 + h * 512 + r * P:
                           sbk * 2048 + h * 512 + (r + 1) * P],
                        qsl, start=True, stop=not diag)
                    if diag:
                        rr = kt_i - qb * 4
                        nc.tensor.matmul(
                            ps_s[:], ident[:],
                            tri[:, rr * 512:(rr + 1) * 512],
                            start=False, stop=True)

                prev_fin = [None]
                at_ref = [None]

                def run_head(qb, h):
                    kmax = (qb + 1) * 4
                    av = aps.tile([P, 512], F32, tag="av", name="av")
                    cs = aps.tile([P, 512], F32, tag="cs", name="cs")
                    ps_s = {}
                    for i in range(min(2, kmax)):
                        ps_s[i] = aps.tile([P, 512], F32, tag="sc", name="ps_s")
                        sc_mm(qb, h, i, ps_s[i])
                    if prev_fin[0] is not None:
                        prev_fin[0]()
                        prev_fin[0] = None
                    for i in range(kmax):
                        zt = ztp.tile([P, 512], F32R, tag="zt", name="zt")
                        nc.scalar.activation(zt[:], ps_s[i][:], EXP, scale=SCALE)
                        del ps_s[i]
                        if i + 2 < kmax:
                            ps_s[i + 2] = aps.tile([P, 512], F32, tag="sc",
                                                   name="ps_s")
                            sc_mm(qb, h, i + 2, ps_s[i + 2])
                        nc.tensor.matmul(
                            av[:],
                            vv[:, i * 512 + h * P: i * 512 + (h + 1) * P],
                            zt[:], start=(i == 0), stop=(i == kmax - 1))
                        nc.tensor.matmul(
                            cs[:], ones[:], zt[:],
                            start=(i == 0), stop=(i == kmax - 1))
                        pop_deferred(1)

                    at_t = at_ref[0]

                    def fin():
                        rec = recp.tile([P, 512], F32, tag="rec", name="rec")
                        nc.vector.reciprocal_approx_fast(rec[:], cs[:])
                        nc.vector.tensor_mul(
                            out=at_t[:, h * 512:(h + 1) * 512],
                            in0=av[:], in1=rec[:])
                    prev_fin[0] = fin

                for qb in range(NQB):
                    at_ref[0] = atp.tile([P, HPG * 512], F32R, tag="at",
                                         name="at_t")
                    at_t_q = at_ref[0]
                    for h in range(HPG):
                        run_head(qb, h)
                    # out-proj for this qb is deferred into the next qb's
                    # attention stream (after its at_t completes)
                    fin_h3 = prev_fin[0]

                    def mk_fin(qb_, at_, f3):
                        def fin2():
                            f3()
                            emit_outproj(qb_, at_)
                        return fin2
                    prev_fin[0] = mk_fin(qb, at_t_q, fin_h3)
                if prev_fin[0] is not None:
                    prev_fin[0]()
                pop_deferred(len(deferred))

    nc.compile()
    return nc


def get_nc():
    if _nc_cache[0] is None:
        _nc_cache[0] = _build()
    return _nc_cache[0]


def make_in_maps(x, wq, wk, wv, wo, freq_pos_enc):
    x = np.asarray(x, np.float32)
    wq = np.asarray(wq, np.float32)
    wk = np.asarray(wk, np.float32)
    wv = np.asarray(wv, np.float32)
    wo = np.asarray(wo, np.float32)
    pe = np.asarray(freq_pos_enc, np.float32)[:S]

    perm = np.concatenate([np.arange(0, DK, 2), np.arange(1, DK, 2)])
    cos = np.ascontiguousarray(np.cos(pe)[:, perm].T)          # [128, S]
    sin = np.ascontiguousarray(np.sin(pe)[:, perm].T)
    sin[:64] *= -1.0

    # tri[r][p, q'] = -1e9 where q' < r*128 + p (strictly-causal mask), else 0
    kk = np.arange(P)[:, None]
    qq = np.arange(512)[None, :]
    tris = np.concatenate(
        [np.where(qq < kk + r * P, -1e9, 0.0).astype(np.float32)
         for r in range(4)], axis=1)

    wq4 = wq.reshape(H, DK, E)[:, perm, :]
    wk4 = wk.reshape(H, DK, E)[:, perm, :]
    wv4 = wv.reshape(H, DK, E)

    in_maps = []
    # xQ[p, e, s] = x[b][s, e*128+p]
    xQb = [np.ascontiguousarray(
        x[b].T.reshape(NE, P, S).transpose(1, 0, 2)) for b in range(B)]
    for c in range(8):
        b, g = c // 4, c % 4
        hs = slice(g * HPG, (g + 1) * HPG)
        in_maps.append({
            "xQ": xQb[b],
            "wqT": np.ascontiguousarray(
                wq4[hs].transpose(2, 0, 1).reshape(E, HD)),
            "wkT": np.ascontiguousarray(
                wk4[hs].transpose(2, 0, 1).reshape(E, HD)),
            "wvT": np.ascontiguousarray(
                wv4[hs].transpose(2, 0, 1).reshape(E, HD)),
            "woT": np.ascontiguousarray(wo[:, g * HD:(g + 1) * HD].T),
            "cosT": cos,
            "sinT": sin,
            "triT": tris,
            "identT": np.eye(P, dtype=np.float32),
            "onesT": np.ones((P, P), np.float32),
        })
    return in_maps


def combine(results):
    out = np.zeros((B, S, E), np.float32)
    for c in range(8):
        out[c // 4] += results[c]["y"]
    return out


def kernel(x, wq, wk, wv, wo, freq_pos_enc, num_heads=None, d_k=None, **_):
    from concourse.bass_utils import run_bass_kernel_spmd
    nc = get_nc()
    in_maps = make_in_maps(x, wq, wk, wv, wo, freq_pos_enc)
    res = run_bass_kernel_spmd(nc, in_maps, core_ids=list(range(8)))
    return combine(res.results)


# revision 18
# speedup vs baseline: 1.0856x; 1.0856x over previous
"""Trainium2 Bass kernel for causal multi-head attention with interleaved RoPE.

Problem: B=2, S=2048, E=2048, H=16, DK=128, fp32, causal, RoPE (interleaved).

Sharding (8 cores): data-parallel over batch (2) x tensor-parallel over head
groups (4 groups of 4 heads). Each core computes, for its (batch b, group g):
    partial_y[S, E] = attn_out_g @ wo[:, g_cols].T
and the host sums the 4 group partials per batch.

Per-core dataflow (all matmuls float32r = full-speed fp32-storage mode):
  - projections in 3 passes (Q, K, V), each pass sb-major with the FULL
    E-contraction accumulated in one PSUM chain (32 matmuls per [128,1024]
    tile) -> a single ACT copy evicts each tile; no DVE eviction adds at
    all.  x is re-DMAed per pass, alternating the scalar/gpsimd queues;
    weights stream on the sync queue.  RoPE (DVE + SBUF-SBUF half-swap
    DMA) runs per s-block right after its eviction.
  - attention per (head, 512-wide q-block), software-pipelined two k-tiles
    deep: scores^T [k,q] on PE into single-bank [128,512] PSUM tiles; causal
    masking via a second accumulating matmul (identity x tri-tile of -1e9)
    so exp(ACT) output needs no post-mask; AV matmuls accumulate on PE while
    the softmax denominator is accumulated OFF the PE by DVE (2/3 of
    k-tiles) and Pool (1/3) elementwise adds, reduced at head end by two
    tiny ones-matmuls; normalize with single-op approx reciprocal + mul.
  - output projection interleaved into the next q-block's score stream via a
    deferred-work queue; wo resident in SBUF (loaded once); PSUM->SBUF
    evictions alternate ACT copy / DVE tensor_scalar_add.
"""
import sys

sys.path.insert(0, "/opt/trn_rl_repo")

import numpy as np

from concourse import bacc, mybir, tile
from concourse import tile_utils

dt = mybir.dt
F32R = dt.float32r
F32 = dt.float32

B, S, E = 2, 2048, 2048
H, DK = 16, 128
HPG = 4            # heads per group
HD = HPG * DK      # 512
P = 128
NE = E // P        # 16 e-tiles
NSB = S // 512     # 4 s-blocks
NQB = 4            # q-blocks
SCALE = 1.0 / float(np.sqrt(DK))

_nc_cache = [None]


def _build():
    # the stock 192KB/partition cap is stale; cayman has 208KB usable
    tile_utils.max_sbuf_usage = 207 * 1024

    nc = bacc.Bacc(None, target_bir_lowering=False)

    xQ = nc.dram_tensor("xQ", [P, NE, S], F32R, kind="ExternalInput")
    wqT = nc.dram_tensor("wqT", [E, HD], F32R, kind="ExternalInput")
    wkT = nc.dram_tensor("wkT", [E, HD], F32R, kind="ExternalInput")
    wvT = nc.dram_tensor("wvT", [E, HD], F32R, kind="ExternalInput")
    woT = nc.dram_tensor("woT", [HD, E], F32R, kind="ExternalInput")
    cosT = nc.dram_tensor("cosT", [P, S], F32, kind="ExternalInput")
    sinT = nc.dram_tensor("sinT", [P, S], F32, kind="ExternalInput")
    triT = nc.dram_tensor("triT", [P, 4 * 512], F32R, kind="ExternalInput")
    identT = nc.dram_tensor("identT", [P, P], F32R, kind="ExternalInput")
    onesT = nc.dram_tensor("onesT", [P, P], F32R, kind="ExternalInput")
    y = nc.dram_tensor("y", [S, E], F32, kind="ExternalOutput")

    EXP = mybir.ActivationFunctionType.Exp
    LN = mybir.ActivationFunctionType.Ln

    with tile.TileContext(nc) as tc:
        with tc.tile_pool(name="res", bufs=1) as res:
            # qt/kt layout: [dk, sb*2048 + h*512 + (s % 512)]
            qt = res.tile([P, HPG * S], F32R, tag="qt")
            kt = res.tile([P, HPG * S], F32R, tag="kt")
            # v layout: [s % 128, (s//128)*512 + h*128 + dv]
            vv = res.tile([P, NE * 512], F32R, tag="vv")
            ones = res.tile([P, P], F32R, tag="ones")
            ident = res.tile([P, P], F32R, tag="ident")
            tri = res.tile([P, 4 * 512], F32R, tag="tri")

            wz = res.tile([P, P], F32, tag="wz")
            nc.vector.memset(wz[:], 0.0)
            nc.scalar.dma_start(out=ones[:], in_=onesT[:, :])

            # ------------- projection phase: 3 passes ---------------------
            with tc.tile_pool(name="csp", bufs=1) as csp, \
                 tc.tile_pool(name="wp", bufs=17) as wp, \
                 tc.tile_pool(name="xsp", bufs=5) as xsp, \
                 tc.tile_pool(name="ropep", bufs=3) as ropep, \
                 tc.tile_pool(name="pps", bufs=4, space="PSUM") as pps:

                # prefetch sb0 x-quads for pass-Q before any bulk constants
                xq0 = {}
                for j in range(4):
                    xq0[j] = xsp.tile([P, 4, 512], F32R, tag="xs", name="xq")
                    eng = nc.scalar if (j & 1) == 0 else nc.gpsimd
                    eng.dma_start(out=xq0[j][:], in_=xQ[:, 4 * j:4 * j + 4, 0:512])

                cos_t = csp.tile([P, S], F32, tag="cos")
                sin_t = csp.tile([P, S], F32, tag="sin")
                nc.sync.dma_start(out=cos_t[:], in_=cosT[:, :])
                nc.sync.dma_start(out=sin_t[:], in_=sinT[:, :])

                def rope(sb, tens, h):
                    # tens slice for (sb, h): u <- u*cos + halfswap(u)*sin_signed
                    base = sb * 2048 + h * 512
                    u = tens[:, base:base + 512]
                    csl = slice(sb * 512, (sb + 1) * 512)
                    eng = nc.vector if h % 2 == 0 else nc.gpsimd
                    sw = ropep.tile([P, 512], F32R, tag="rp", name="sw")
                    nc.sync.dma_start(out=sw[0:64, :], in_=u[64:128, :])
                    nc.sync.dma_start(out=sw[64:128, :], in_=u[0:64, :])
                    eng.tensor_mul(out=sw[:], in0=sw[:], in1=sin_t[:, csl])
                    eng.tensor_mul(out=u, in0=u, in1=cos_t[:, csl])
                    eng.tensor_add(out=u, in0=u, in1=sw[:])

                first = [True]

                def qk_pass(wsrc, dest, pre=None):
                    w_t = {}
                    for e in range(NE):
                        w_t[e] = wp.tile([P, HD], F32R, tag="w", name="w_t")
                        nc.sync.dma_start(
                            out=w_t[e][:], in_=wsrc[e * P:(e + 1) * P, :])
                    for sb in range(NSB):
                        if sb == 0 and pre is not None:
                            xq = pre
                        else:
                            xq = {}
                            for j in range(4):
                                xq[j] = xsp.tile([P, 4, 512], F32R, tag="xs",
                                                 name="xq")
                                eng = nc.scalar if (j & 1) == 0 else nc.gpsimd
                                eng.dma_start(
                                    out=xq[j][:],
                                    in_=xQ[:, 4 * j:4 * j + 4,
                                           sb * 512:(sb + 1) * 512])
                        xs_t = {e: xq[e // 4][:, e % 4, :] for e in range(NE)}
                        ps = [pps.tile([P, 1024], F32, tag="ps", name="psqk")
                              for _ in range(2)]
                        if first[0]:
                            first[0] = False
                            for _ in range(24):
                                nc.tensor.matmul(ps[0][0:32, 0:128],
                                                 wz[:, 0:32], wz[:],
                                                 start=True, stop=True)
                        for e in range(NE):
                            st_, sp_ = e == 0, e == NE - 1
                            for h in range(HPG):
                                nc.tensor.matmul(
                                    ps[h // 2][:, (h % 2) * 512:(h % 2) * 512 + 512],
                                    w_t[e][:, h * P:(h + 1) * P],
                                    xs_t[e][:], start=st_, stop=sp_)
                        for hp in range(2):
                            nc.scalar.copy(
                                out=dest[:, sb * 2048 + hp * 1024:
                                         sb * 2048 + hp * 1024 + 1024],
                                in_=ps[hp][:])
                        for h in range(HPG):
                            rope(sb, dest, h)

                qk_pass(wqT, qt, pre=xq0)
                qk_pass(wkT, kt)

                # V pass
                wv_t = {}
                for e in range(NE):
                    wv_t[e] = wp.tile([P, HD], F32R, tag="w", name="wv_t")
                    nc.sync.dma_start(
                        out=wv_t[e][:], in_=wvT[e * P:(e + 1) * P, :])
                for sb in range(NSB):
                    xq = {}
                    for j in range(4):
                        xq[j] = xsp.tile([P, 4, 512], F32R, tag="xs", name="xq")
                        eng = nc.scalar if (j & 1) == 0 else nc.gpsimd
                        eng.dma_start(
                            out=xq[j][:],
                            in_=xQ[:, 4 * j:4 * j + 4,
                                   sb * 512:(sb + 1) * 512])
                    xs_t = {e: xq[e // 4][:, e % 4, :] for e in range(NE)}
                    psv = [pps.tile([P, 1024], F32, tag="ps", name="psv")
                           for _ in range(2)]
                    for e in range(NE):
                        st_, sp_ = e == 0, e == NE - 1
                        for st in range(4):
                            nc.tensor.matmul(
                                psv[st // 2][:, (st % 2) * 512:(st % 2) * 512 + 512],
                                xs_t[e][:, st * P:(st + 1) * P],
                                wv_t[e][:], start=st_, stop=sp_)
                    for sp2 in range(2):
                        gst = sb * 4 + sp2 * 2
                        nc.scalar.copy(out=vv[:, gst * 512:(gst + 2) * 512],
                                       in_=psv[sp2][:])
                    if sb == 0:
                        nc.gpsimd.dma_start(out=ident[:], in_=identT[:, :])
                        nc.gpsimd.dma_start(out=tri[:], in_=triT[:, :])

            # ------------- attention + out-proj phase --------------------
            with tc.tile_pool(name="worp", bufs=1) as worp, \
                 tc.tile_pool(name="atp", bufs=2) as atp, \
                 tc.tile_pool(name="ztp", bufs=4) as ztp, \
                 tc.tile_pool(name="recp", bufs=2) as recp, \
                 tc.tile_pool(name="obp", bufs=4) as obp, \
                 tc.tile_pool(name="aps", bufs=2, space="PSUM") as aps:

                wo_r = []
                for hh in range(HPG):
                    wt = worp.tile([P, E], F32R, tag=f"wo{hh}", name="wt")
                    nc.sync.dma_start(out=wt[:], in_=woT[hh * P:(hh + 1) * P, :])
                    wo_r.append(wt)

                # deferred out-proj tiles, popped between attention items
                deferred = []

                def pop_deferred(n=1):
                    for _ in range(n):
                        if deferred:
                            deferred.pop(0)()

                def emit_outproj(qb, at_t):
                    # 16 psum tiles [128 q, 512 e], each = 4 accumulating MMs
                    def mk(st, e5):
                        def go():
                            ps_o = aps.tile([P, 512], F32, tag="po", name="ps_o")
                            for h in range(HPG):
                                nc.tensor.matmul(
                                    ps_o[:],
                                    at_t[:, h * 512 + st * P: h * 512 + (st + 1) * P],
                                    wo_r[h][:, e5 * 512:(e5 + 1) * 512],
                                    start=(h == 0), stop=(h == HPG - 1))
                            ob = obp.tile([P, 512], F32, tag="ob", name="ob")
                            nc.vector.tensor_scalar_add(ob[:], ps_o[:], 0.0)
                            srow = qb * 512 + st * P
                            nc.sync.dma_start(
                                out=y[srow:srow + P, e5 * 512:(e5 + 1) * 512],
                                in_=ob[:])
                        return go
                    for st in range(4):
                        for e5 in range(4):
                            deferred.append(mk(st, e5))

                def sc_mm(qb, h, kt_i, ps_s):
                    # scores^T for one 128-wide k-tile, plus causal tri-mask.
                    # Diagonal tiles r>=1 only need q' >= r*128: compute the
                    # [r*128:512] slice (columns below are fully masked).
                    sbk, r = divmod(kt_i, 4)
                    diag = kt_i >= qb * 4
                    off = (kt_i - qb * 4) * P if diag else 0
                    qsl = qt[:, qb * 2048 + h * 512 + off:
                             qb * 2048 + (h + 1) * 512]
                    nc.tensor.matmul(
                        ps_s[:, off:512],
                        kt[:, sbk * 2048 + h * 512 + r * P:
                           sbk * 2048 + h * 512 + (r + 1) * P],
                        qsl, start=True, stop=not diag)
                    if diag:
                        rr = kt_i - qb * 4
                        nc.tensor.matmul(
                            ps_s[:, off:512], ident[:],
                            tri[:, rr * 512 + off:(rr + 1) * 512],
                            start=False, stop=True)
                    return off

                prev_fin = [None]
                at_ref = [None]

                def run_head(qb, h):
                    kmax = (qb + 1) * 4
                    av = aps.tile([P, 512], F32, tag="av", name="av")
                    cs = aps.tile([P, 512], F32, tag="cs", name="cs")
                    ps_s, offs = {}, {}
                    for i in range(min(2, kmax)):
                        ps_s[i] = aps.tile([P, 512], F32, tag="sc", name="ps_s")
                        offs[i] = sc_mm(qb, h, i, ps_s[i])
                    if prev_fin[0] is not None:
                        prev_fin[0]()
                        prev_fin[0] = None
                    for i in range(kmax):
                        o = offs[i]
                        zt = ztp.tile([P, 512], F32R, tag="zt", name="zt")
                        nc.scalar.activation(zt[:, o:512], ps_s[i][:, o:512],
                                             EXP, scale=SCALE)
                        del ps_s[i]
                        if i + 2 < kmax:
                            ps_s[i + 2] = aps.tile([P, 512], F32, tag="sc",
                                                   name="ps_s")
                            offs[i + 2] = sc_mm(qb, h, i + 2, ps_s[i + 2])
                        nc.tensor.matmul(
                            av[:, o:512],
                            vv[:, i * 512 + h * P: i * 512 + (h + 1) * P],
                            zt[:, o:512], start=(i == 0), stop=(i == kmax - 1))
                        nc.tensor.matmul(
                            cs[:, o:512], ones[:], zt[:, o:512],
                            start=(i == 0), stop=(i == kmax - 1))
                        pop_deferred(1)

                    at_t = at_ref[0]

                    def fin():
                        rec = recp.tile([P, 512], F32, tag="rec", name="rec")
                        nc.vector.reciprocal_approx_fast(rec[:], cs[:])
                        nc.vector.tensor_mul(
                            out=at_t[:, h * 512:(h + 1) * 512],
                            in0=av[:], in1=rec[:])
                    prev_fin[0] = fin

                for qb in range(NQB):
                    at_ref[0] = atp.tile([P, HPG * 512], F32R, tag="at",
                                         name="at_t")
                    at_t_q = at_ref[0]
                    for h in range(HPG):
                        run_head(qb, h)
                    # out-proj for this qb is deferred into the next qb's
                    # attention stream (after its at_t completes)
                    fin_h3 = prev_fin[0]

                    def mk_fin(qb_, at_, f3):
                        def fin2():
                            f3()
                            emit_outproj(qb_, at_)
                        return fin2
                    prev_fin[0] = mk_fin(qb, at_t_q, fin_h3)
                if prev_fin[0] is not None:
                    prev_fin[0]()
                pop_deferred(len(deferred))

    nc.compile()
    return nc


def get_nc():
    if _nc_cache[0] is None:
        _nc_cache[0] = _build()
    return _nc_cache[0]


def make_in_maps(x, wq, wk, wv, wo, freq_pos_enc):
    x = np.asarray(x, np.float32)
    wq = np.asarray(wq, np.float32)
    wk = np.asarray(wk, np.float32)
    wv = np.asarray(wv, np.float32)
    wo = np.asarray(wo, np.float32)
    pe = np.asarray(freq_pos_enc, np.float32)[:S]

    perm = np.concatenate([np.arange(0, DK, 2), np.arange(1, DK, 2)])
    cos = np.ascontiguousarray(np.cos(pe)[:, perm].T)          # [128, S]
    sin = np.ascontiguousarray(np.sin(pe)[:, perm].T)
    sin[:64] *= -1.0

    # tri[r][p, q'] = -1e9 where q' < r*128 + p (strictly-causal mask), else 0
    kk = np.arange(P)[:, None]
    qq = np.arange(512)[None, :]
    tris = np.concatenate(
        [np.where(qq < kk + r * P, -1e9, 0.0).astype(np.float32)
         for r in range(4)], axis=1)

    wq4 = wq.reshape(H, DK, E)[:, perm, :]
    wk4 = wk.reshape(H, DK, E)[:, perm, :]
    wv4 = wv.reshape(H, DK, E)

    in_maps = []
    # xQ[p, e, s] = x[b][s, e*128+p]
    xQb = [np.ascontiguousarray(
        x[b].T.reshape(NE, P, S).transpose(1, 0, 2)) for b in range(B)]
    for c in range(8):
        b, g = c // 4, c % 4
        hs = slice(g * HPG, (g + 1) * HPG)
        in_maps.append({
            "xQ": xQb[b],
            "wqT": np.ascontiguousarray(
                wq4[hs].transpose(2, 0, 1).reshape(E, HD)),
            "wkT": np.ascontiguousarray(
                wk4[hs].transpose(2, 0, 1).reshape(E, HD)),
            "wvT": np.ascontiguousarray(
                wv4[hs].transpose(2, 0, 1).reshape(E, HD)),
            "woT": np.ascontiguousarray(wo[:, g * HD:(g + 1) * HD].T),
            "cosT": cos,
            "sinT": sin,
            "triT": tris,
            "identT": np.eye(P, dtype=np.float32),
            "onesT": np.ones((P, P), np.float32),
        })
    return in_maps


def combine(results):
    out = np.zeros((B, S, E), np.float32)
    for c in range(8):
        out[c // 4] += results[c]["y"]
    return out


def kernel(x, wq, wk, wv, wo, freq_pos_enc, num_heads=None, d_k=None, **_):
    from concourse.bass_utils import run_bass_kernel_spmd
    nc = get_nc()
    in_maps = make_in_maps(x, wq, wk, wv, wo, freq_pos_enc)
    res = run_bass_kernel_spmd(nc, in_maps, core_ids=list(range(8)))
    return combine(res.results)
